# revision 6
# baseline (speedup 1.0000x reference)
"""Trainium2 Bass kernel for CrossAttention (B=2, T=S=2048, E=1024, H=16, D=64).

Sharding: 8 cores = 2 (batch) x 4 (head groups of 4 heads).
Each core computes, for its (b, g):
  - Q/K projections in feature-major layout: QT/KT = [256, 2048]
  - V projection in sequence-major layout with an appended ones column per
    head (gives the softmax denominator for free from the attn@V matmul)
  - causal flash-style attention (exp on ScalarE with the 1/sqrt(d) scale
    folded in; diagonal-block triangle masked via a fp16 multiply)
  - output projection partial: outT_partial = Wo[:, group].T-style [1024, 2048]
Host: shards/transposes inputs, gathers partials, sums over the 4 groups per
batch and adds bo.

Schedule: fully software-pipelined over t-chunks of 512 --
  proj(j) -> attn(h0,j) + attn(h1,j) -> oproj(j)
with proj(j+1) emitted between attn(h1,j) and oproj(j) so the PE never idles
while the (h1,j) softmax-normalize chain completes.  Causal boundary blocks
are computed at narrowed width (cols >= 128*r within the chunk) on PE/ACT/DVE.
Bulk loads (x tiles batched 4 k-blocks per DMA) and stores issue on the sync
queue; small SBUF-to-SBUF hops and partition broadcasts ride gpsimd.
"""

import ml_dtypes
import numpy as np

import concourse.bass as bass
import concourse.bacc as bacc
import concourse.mybir as mybir
import concourse.tile as tile
from concourse.bass_utils import run_bass_kernel_spmd

P = 128
T = 2048          # target length
S = 2048          # source length
E = 1024          # embed dim
D = 64            # head dim
GC = 256          # channels per group (4 heads * 64)
NHL = 4           # heads per core (local)
KB = E // P       # 8 full k-blocks for the E contraction
TJ = 512          # t-chunk width
NTJ = T // TJ     # 4
NSB = S // P      # 16 s-blocks
VC = NHL * (D + 1)  # 260 = V-projection cols (64 V + 1 ones per head)
SCALE = float(D) ** -0.5  # 0.125

F32 = mybir.dt.float32
BF16 = mybir.dt.bfloat16
F16 = mybir.dt.float16

# fp16 runs the PE at 1 cyc/row (vs fp32's 2 half-speed passes) with an
# 11-bit mantissa; all tensors here are O(1)-scaled so range is safe.
DT = F16


def _build_program():
    nc = bacc.Bacc()

    xq = nc.dram_tensor("xq_t", [E, T], DT, kind="ExternalInput")
    xk = nc.dram_tensor("xk_t", [E, S], DT, kind="ExternalInput")
    xv = nc.dram_tensor("xv_t", [E, S], DT, kind="ExternalInput")
    wq = nc.dram_tensor("wq_t", [E, GC], DT, kind="ExternalInput")
    wk = nc.dram_tensor("wk_t", [E, GC], DT, kind="ExternalInput")
    wv = nc.dram_tensor("wv_t", [E + 1, VC], DT, kind="ExternalInput")
    wo = nc.dram_tensor("wo_t", [GC, E], DT, kind="ExternalInput")
    # fp16 lower-triangular keep-mask [s, t] = (t >= s) for diagonal blocks,
    # and an f32 tensor holding key-padding columns plus per-channel q/k biases
    tri = nc.dram_tensor("tri", [P, P], DT, kind="ExternalInput")
    padb = nc.dram_tensor("padb", [P, NSB + 4], F32, kind="ExternalInput")
    out_t = nc.dram_tensor("out_t", [E, T], F16, kind="ExternalOutput")

    wk_r = wk.rearrange("(kb p) c -> p kb c", p=P)
    wq_r = wq.rearrange("(kb p) c -> p kb c", p=P)
    wo_r = wo.rearrange("(cc p) o -> p cc o", p=P)

    xt = {}
    with tile.TileContext(nc) as tc:
        with (
            tc.tile_pool(name="consts", bufs=1) as cpool,
            tc.tile_pool(name="xs", bufs=12) as xpool,
            tc.tile_pool(name="persist", bufs=1) as ppool,
            tc.tile_pool(name="expw", bufs=4) as epool,
            tc.tile_pool(name="ao", bufs=1) as apool,
            tc.tile_pool(name="aon", bufs=2) as npool,
            tc.tile_pool(name="ft", bufs=2) as fpool,
            tc.tile_pool(name="ps", bufs=1, space="PSUM") as pspool,
        ):
            # ---- SBUF homes for weights / constants ----
            wq_sb = cpool.tile([P, KB, GC], DT, name="wq_sb")
            wk_sb = cpool.tile([P, KB, GC], DT, name="wk_sb")
            wv_sb = cpool.tile([P, KB + 1, VC], DT, name="wv_sb")
            wo_sb = cpool.tile([P, 2, E], DT, name="wo_sb")
            tri_sb = cpool.tile([P, P], DT, name="tri_sb")
            padb_sb = cpool.tile([P, NSB + 4], F32, name="padb_sb")
            ones_sb = cpool.tile([1, P], DT, name="ones_sb")

            # ---- persistent activations ----
            qt_sb = ppool.tile([P, 2, T], DT, name="qt_sb")
            kt_sb = ppool.tile([P, 2, S], DT, name="kt_sb")
            v_sb = ppool.tile([P, NSB, VC], DT, name="v_sb")

            # ---- DMA issue helpers: x tiles in 4-kb batches on sync ----
            def dma_x_batch(nm, dram, j, kq):
                t_ = xpool.tile([P, 4, TJ], DT, tag="xs", name=f"t{nm}")
                nc.sync.dma_start(
                    t_[:],
                    dram.rearrange("(kb p) t -> p kb t", p=P)[
                        :, 4 * kq : 4 * (kq + 1), j * TJ : (j + 1) * TJ
                    ],
                )
                for q in range(4):
                    xt[(nm, 4 * kq + q, j)] = t_[:, q, :]

            def dma_x_step(j):
                for nm, dram in (("xk", xk), ("xq", xq), ("xv", xv)):
                    for kq in range(2):
                        dma_x_batch(nm, dram, j, kq)

            # startup: interleave first-needed weights with first x tiles
            nc.sync.dma_start(wk_sb[:, :4, :], wk_r[:, :4, :])
            dma_x_batch("xk", xk, 0, 0)
            nc.sync.dma_start(wk_sb[:, 4:, :], wk_r[:, 4:, :])
            dma_x_batch("xk", xk, 0, 1)
            nc.sync.dma_start(padb_sb[:], padb[:])
            nc.sync.dma_start(wq_sb[:], wq_r)
            dma_x_batch("xq", xq, 0, 0)
            dma_x_batch("xq", xq, 0, 1)
            nc.sync.dma_start(
                wv_sb[:, :KB, :],
                wv[: KB * P, :].rearrange("(kb p) c -> p kb c", p=P),
            )
            nc.sync.dma_start(wv_sb[0:1, KB, :], wv[KB * P : KB * P + 1, :])
            dma_x_batch("xv", xv, 0, 0)
            dma_x_batch("xv", xv, 0, 1)
            nc.sync.dma_start(tri_sb[:], tri[:])
            nc.any.memset(ones_sb[:], 1.0)

            # ---- per-chunk emission bodies ----
            def proj_step(j):
                jsl = slice(j * TJ, (j + 1) * TJ)
                # K then Q, channel-major [256ch, t]
                for nm, w_sb, dst, bi in (
                    ("xk", wk_sb, kt_sb, 1),
                    ("xq", wq_sb, qt_sb, 0),
                ):
                    for mc in range(2):
                        ps = pspool.tile([P, TJ], F32, tag="ps_pr", name="ps_pr", bufs=2)
                        for kb in range(KB):
                            nc.tensor.matmul(
                                ps[:],
                                lhsT=w_sb[:, kb, mc * P : (mc + 1) * P],
                                rhs=xt[(nm, kb, j)][:],
                                start=(kb == 0),
                                stop=(kb == KB - 1),
                            )
                        nc.vector.tensor_scalar_add(
                            dst[:, mc, jsl],
                            ps[:],
                            padb_sb[:, NSB + 2 * bi + mc : NSB + 2 * bi + mc + 1],
                        )
                # V, sequence-major [s, 260] with ones column per head
                for ii in range(TJ // P):
                    i = j * (TJ // P) + ii
                    ps = pspool.tile([P, TJ], F32, tag="ps_pr", name="ps_v", bufs=2)
                    for kb in range(KB):
                        nc.tensor.matmul(
                            ps[:, :VC],
                            lhsT=xt[("xv", kb, j)][:, ii * P : (ii + 1) * P],
                            rhs=wv_sb[:, kb, :],
                            start=(kb == 0),
                            stop=False,
                        )
                    nc.tensor.matmul(
                        ps[:, :VC],
                        lhsT=ones_sb[0:1, 0:P],
                        rhs=wv_sb[0:1, KB, :],
                        start=False,
                        stop=True,
                    )
                    nc.vector.tensor_scalar_mul(
                        v_sb[:, i, :], ps[:, :VC], padb_sb[:, i : i + 1]
                    )

            def attn_chunk(hp, j, aoTn):
                """Causal attention for head pair hp over t-chunk j.

                Boundary s-blocks (i = 4j..4j+3, r = i-4j) are narrowed to
                cols >= 128*r of the chunk on scores/exp/attnV; the diagonal
                128x128 triangle is masked with a fp16 multiply on DVE.
                """
                nsb_j = 4 * j + 4
                jsl = slice(j * TJ, (j + 1) * TJ)
                av_ps = [
                    pspool.tile([P, TJ], F32, tag=f"ps_av{lh}", name="ps_av", bufs=1)
                    for lh in range(2)
                ]
                ets = {}

                def emit_scores_pair(m):
                    ps2s = [
                        pspool.tile([P, 2, TJ], F32, tag="ps_sc", name="ps_sc", bufs=2)
                        for _ in range(2)
                    ]
                    for u in range(2):
                        i = 2 * m + u
                        r = i - 4 * j
                        off = 128 * r if r > 0 else 0
                        for lh in range(2):
                            base = D * lh
                            nc.tensor.matmul(
                                ps2s[lh][:, u, off:],
                                lhsT=kt_sb[base : base + D, hp, i * P : (i + 1) * P],
                                rhs=qt_sb[base : base + D, hp, j * TJ + off : (j + 1) * TJ],
                                start=True,
                                stop=True,
                            )
                    r1 = 2 * m + 1 - 4 * j  # r of u=1; >0 means narrowed pair
                    for lh in range(2):
                        et2 = epool.tile([P, 2, TJ], DT, tag="exp", name="et2")
                        if r1 <= 0:
                            nc.scalar.activation(
                                et2[:],
                                ps2s[lh][:],
                                mybir.ActivationFunctionType.Exp,
                                scale=SCALE,
                            )
                        else:
                            for u in range(2):
                                off = 128 * (r1 - 1 + u)
                                nc.scalar.activation(
                                    et2[:, u, off:],
                                    ps2s[lh][:, u, off:],
                                    mybir.ActivationFunctionType.Exp,
                                    scale=SCALE,
                                )
                        for u in range(2):
                            r = 2 * m + u - 4 * j
                            if r >= 0:
                                # diagonal block: mask the 128x128 triangle
                                nc.vector.tensor_mul(
                                    out=et2[:, u, 128 * r : 128 * (r + 1)],
                                    in0=et2[:, u, 128 * r : 128 * (r + 1)],
                                    in1=tri_sb[:],
                                )
                        ets[(m, lh)] = et2

                def emit_av_pair(m, lh):
                    h65 = (hp * 2 + lh) * (D + 1)
                    et2 = ets.pop((m, lh))
                    for u in range(2):
                        i = 2 * m + u
                        r = i - 4 * j
                        off = 128 * r if r > 0 else 0
                        nc.tensor.matmul(
                            av_ps[lh][: D + 1, off:],
                            lhsT=v_sb[:, i, h65 : h65 + D + 1],
                            rhs=et2[:, u, off:],
                            start=(i == 0),
                            stop=(i == nsb_j - 1),
                            skip_group_check=True,
                        )

                # software pipeline: attn@V trails scores/exp by one pair
                npairs = nsb_j // 2
                for m in range(npairs):
                    emit_scores_pair(m)
                    if m >= 1:
                        for lh in range(2):
                            emit_av_pair(m - 1, lh)
                for lh in range(2):
                    emit_av_pair(npairs - 1, lh)
                # normalize: den sits in row D of av_ps; reciprocal in place,
                # broadcast to 64 partitions, multiply.  lh=1 lands on
                # partitions 64-127 of aoTn via an SBUF->SBUF shift DMA.
                for lh in range(2):
                    aoTS = apool.tile(
                        [D + 1, TJ], F32, tag=f"aoTS{lh}", name="aoTS", bufs=2
                    )
                    nc.any.tensor_copy(out=aoTS[:], in_=av_ps[lh][: D + 1, :])
                    rcp = npool.tile([1, TJ], F32, tag="rcp", name="rcp", bufs=3)
                    nc.gpsimd.dma_start(rcp[:], aoTS[D : D + 1, :])
                    nc.vector.reciprocal_approx_fast(rcp[:], rcp[:])
                    rb64 = npool.tile([D, TJ], F32, tag="rb64", name="rb64", bufs=3)
                    nc.gpsimd.partition_broadcast(rb64[:], rcp[0:1, :])
                    if lh == 0:
                        nc.vector.tensor_mul(
                            out=aoTn[0:D, :], in0=aoTS[0:D, :], in1=rb64[:]
                        )
                    else:
                        tmp = npool.tile([D, TJ], DT, tag="aon", name="aon", bufs=3)
                        nc.vector.tensor_mul(
                            out=tmp[:], in0=aoTS[0:D, :], in1=rb64[:]
                        )
                        # partition shift 0-63 -> 64-127 via SBUF DMA
                        nc.gpsimd.dma_start(aoTn[D : 2 * D, :], tmp[:])

            def oproj_step(j, aoTn0, aoTn1):
                jsl = slice(j * TJ, (j + 1) * TJ)
                for mc in range(KB):
                    ps = pspool.tile([P, TJ], F32, tag="ps_pr", name="ps_o", bufs=2)
                    for cc, src in ((0, aoTn0), (1, aoTn1)):
                        nc.tensor.matmul(
                            ps[:],
                            lhsT=wo_sb[:, cc, mc * P : (mc + 1) * P],
                            rhs=src[:],
                            start=(cc == 0),
                            stop=(cc == 1),
                        )
                    oc = fpool.tile([P, TJ], F16, tag="oc", name="oc", bufs=4)
                    nc.any.tensor_copy(out=oc[:], in_=ps[:])
                    nc.sync.dma_start(
                        out_t[mc * P : (mc + 1) * P, jsl], oc[:]
                    )

            # ---- main pipeline ----
            proj_step(0)
            for j in range(NTJ):
                if j == 0:
                    nc.sync.dma_start(wo_sb[:], wo_r)
                if j + 1 < NTJ:
                    dma_x_step(j + 1)
                aoTns = []
                for hp in range(2):
                    aoTn = npool.tile([P, TJ], DT, tag=f"aoTn{hp}", name="aoTn", bufs=2)
                    attn_chunk(hp, j, aoTn)
                    aoTns.append(aoTn)
                if j + 1 < NTJ:
                    proj_step(j + 1)
                oproj_step(j, aoTns[0], aoTns[1])

    nc.compile()
    return nc


_NC_CACHE = None


def _get_nc():
    global _NC_CACHE
    if _NC_CACHE is None:
        _NC_CACHE = _build_program()
    return _NC_CACHE


def _make_in_maps(query, key, value, key_padding_mask, Wq, bq, Wk, bk, Wv, bv, Wo, bo):
    f32 = np.float32
    query = np.asarray(query, f32)
    key = np.asarray(key, f32)
    value = np.asarray(value, f32)
    kpm = np.asarray(key_padding_mask, bool)
    Wq, bq = np.asarray(Wq, f32), np.asarray(bq, f32)
    Wk, bk = np.asarray(Wk, f32), np.asarray(bk, f32)
    Wv, bv = np.asarray(Wv, f32), np.asarray(bv, f32)
    Wo = np.asarray(Wo, f32)

    # constants shared by all cores
    f16 = np.float16
    tri_np = (np.arange(P)[None, :] >= np.arange(P)[:, None]).astype(f16)

    in_maps = []
    for c in range(8):
        b, g = divmod(c, 4)
        cols = slice(g * GC, (g + 1) * GC)

        wq_t = Wq[cols, :].T.astype(f16)
        wk_t = Wk[cols, :].T.astype(f16)

        wv_t = np.zeros((E + 1, VC), f16)
        for h in range(NHL):
            ch = slice(g * GC + h * D, g * GC + (h + 1) * D)
            wv_t[:E, h * (D + 1) : h * (D + 1) + D] = Wv[ch, :].T
            wv_t[E, h * (D + 1) : h * (D + 1) + D] = bv[ch]
            wv_t[E, h * (D + 1) + D] = 1.0  # ones column -> softmax denominator

        wo_t = np.ascontiguousarray(Wo[:, cols].T.astype(f16))

        padb_np = np.where(kpm[b], 0.0, 1.0).astype(f32).reshape(NSB, P).T
        biases = np.stack(
            [bq[cols][:P], bq[cols][P:], bk[cols][:P], bk[cols][P:]], axis=1
        ).astype(f32)
        padb_np = np.ascontiguousarray(np.concatenate([padb_np, biases], axis=1))

        in_maps.append(
            {
                "xq_t": np.ascontiguousarray(query[b].T.astype(f16)),
                "xk_t": np.ascontiguousarray(key[b].T.astype(f16)),
                "xv_t": np.ascontiguousarray(value[b].T.astype(f16)),
                "wq_t": np.ascontiguousarray(wq_t),
                "wk_t": np.ascontiguousarray(wk_t),
                "wv_t": wv_t,
                "wo_t": wo_t,
                "tri": tri_np,
                "padb": padb_np,
            }
        )
    return in_maps


def kernel(**inputs) -> np.ndarray:
    nc = _get_nc()
    in_maps = _make_in_maps(**inputs)
    res = run_bass_kernel_spmd(nc, in_maps, core_ids=list(range(8)))
    bo = np.asarray(inputs["bo"], np.float32)
    B = inputs["query"].shape[0]
    out = np.zeros((B, T, E), np.float32)
    for c in range(8):
        b = c // 4
        out[b] += res.results[c]["out_t"].T.astype(np.float32)
    out += bo[None, None, :]
    return out


# revision 8
# speedup vs baseline: 1.0642x; 1.0642x over previous
"""Trainium2 Bass kernel for CrossAttention (B=2, T=S=2048, E=1024, H=16, D=64).

Sharding: 8 cores = 2 (batch) x 4 (head groups of 4 heads).
Each core computes, for its (b, g):
  - Q/K projections in feature-major layout: QT/KT = [256, 2048]
  - V projection in sequence-major layout with an appended ones column per
    head (gives the softmax denominator for free from the attn@V matmul)
  - causal flash-style attention (exp on ScalarE with the 1/sqrt(d) scale
    folded in; diagonal-block triangle masked via a fp16 multiply)
  - output projection partial: outT_partial = Wo[:, group].T-style [1024, 2048]
Host: shards/transposes inputs, gathers partials, sums over the 4 groups per
batch and adds bo.

Schedule: fully software-pipelined over t-chunks of 512 --
  proj(j) -> attn(h0,j) + attn(h1,j) -> oproj(j)
with proj(j+1) emitted between attn(h1,j) and oproj(j) so the PE never idles
while the (h1,j) softmax-normalize chain completes.  Causal boundary blocks
are computed at narrowed width (cols >= 128*r within the chunk) on PE/ACT/DVE.
Bulk loads (x tiles batched 4 k-blocks per DMA) and stores issue on the sync
queue; small SBUF-to-SBUF hops and partition broadcasts ride gpsimd.
"""

import ml_dtypes
import numpy as np

import concourse.bass as bass
import concourse.bacc as bacc
import concourse.mybir as mybir
import concourse.tile as tile
from concourse.bass_utils import run_bass_kernel_spmd

P = 128
T = 2048          # target length
S = 2048          # source length
E = 1024          # embed dim
D = 64            # head dim
GC = 256          # channels per group (4 heads * 64)
NHL = 4           # heads per core (local)
KB = E // P       # 8 full k-blocks for the E contraction
TJ = 512          # t-chunk width
NTJ = T // TJ     # 4
NSB = S // P      # 16 s-blocks
VC = NHL * (D + 1)  # 260 = V-projection cols (64 V + 1 ones per head)
SCALE = float(D) ** -0.5  # 0.125

F32 = mybir.dt.float32
BF16 = mybir.dt.bfloat16
F16 = mybir.dt.float16

# fp16 runs the PE at 1 cyc/row (vs fp32's 2 half-speed passes) with an
# 11-bit mantissa; all tensors here are O(1)-scaled so range is safe.
DT = F16


def _build_program():
    nc = bacc.Bacc()

    xq = nc.dram_tensor("xq_t", [E, T], DT, kind="ExternalInput")
    xk = nc.dram_tensor("xk_t", [E, S], DT, kind="ExternalInput")
    xv = nc.dram_tensor("xv_t", [E, S], DT, kind="ExternalInput")
    wq = nc.dram_tensor("wq_t", [E, GC], DT, kind="ExternalInput")
    wk = nc.dram_tensor("wk_t", [E, GC], DT, kind="ExternalInput")
    wv = nc.dram_tensor("wv_t", [E + 1, VC], DT, kind="ExternalInput")
    wo = nc.dram_tensor("wo_t", [GC, E], DT, kind="ExternalInput")
    # fp16 lower-triangular keep-mask [s, t] = (t >= s) for diagonal blocks,
    # and an f32 tensor holding key-padding columns plus per-channel q/k biases
    tri = nc.dram_tensor("tri", [P, P], DT, kind="ExternalInput")
    padb = nc.dram_tensor("padb", [P, NSB + 4 + VC], F32, kind="ExternalInput")
    out_t = nc.dram_tensor("out_t", [E, T], F16, kind="ExternalOutput")

    wk_r = wk.rearrange("(kb p) c -> p kb c", p=P)
    wq_r = wq.rearrange("(kb p) c -> p kb c", p=P)
    wo_r = wo.rearrange("(cc p) o -> p cc o", p=P)

    xt = {}
    with tile.TileContext(nc) as tc:
        with (
            tc.tile_pool(name="consts", bufs=1) as cpool,
            tc.tile_pool(name="xs", bufs=12) as xpool,
            tc.tile_pool(name="persist", bufs=1) as ppool,
            tc.tile_pool(name="expw", bufs=4) as epool,
            tc.tile_pool(name="ao", bufs=1) as apool,
            tc.tile_pool(name="aon", bufs=2) as npool,
            tc.tile_pool(name="ft", bufs=2) as fpool,
            tc.tile_pool(name="ps", bufs=1, space="PSUM") as pspool,
        ):
            # ---- SBUF homes for weights / constants ----
            wq_sb = cpool.tile([P, KB, GC], DT, name="wq_sb")
            wk_sb = cpool.tile([P, KB, GC], DT, name="wk_sb")
            wv_sb = cpool.tile([P, KB, VC], DT, name="wv_sb")
            wo_sb = cpool.tile([P, 2, E], DT, name="wo_sb")
            tri_sb = cpool.tile([P, P], DT, name="tri_sb")
            padb_sb = cpool.tile([P, NSB + 4 + VC], F32, name="padb_sb")
            warm_sb = cpool.tile([1, 8], F32, name="warm_sb")

            # ---- persistent activations ----
            qt_sb = ppool.tile([P, 2, T], DT, name="qt_sb")
            kt_sb = ppool.tile([P, 2, S], DT, name="kt_sb")
            v_sb = ppool.tile([P, NSB, VC], DT, name="v_sb")

            # ---- DMA issue helpers: x tiles in 4-kb batches on sync ----
            def dma_x_batch(nm, dram, j, kq):
                t_ = xpool.tile([P, 4, TJ], DT, tag="xs", name=f"t{nm}")
                nc.sync.dma_start(
                    t_[:],
                    dram.rearrange("(kb p) t -> p kb t", p=P)[
                        :, 4 * kq : 4 * (kq + 1), j * TJ : (j + 1) * TJ
                    ],
                )
                for q in range(4):
                    xt[(nm, 4 * kq + q, j)] = t_[:, q, :]

            def dma_x_step(j):
                for nm, dram in (("xk", xk), ("xq", xq), ("xv", xv)):
                    for kq in range(2):
                        dma_x_batch(nm, dram, j, kq)

            # startup: interleave first-needed weights with first x tiles
            nc.sync.dma_start(wk_sb[:, :4, :], wk_r[:, :4, :])
            dma_x_batch("xk", xk, 0, 0)
            nc.sync.dma_start(wk_sb[:, 4:, :], wk_r[:, 4:, :])
            dma_x_batch("xk", xk, 0, 1)
            nc.sync.dma_start(padb_sb[:], padb[:])
            nc.sync.dma_start(wq_sb[:], wq_r)
            dma_x_batch("xq", xq, 0, 0)
            dma_x_batch("xq", xq, 0, 1)
            nc.sync.dma_start(
                wv_sb[:],
                wv[: KB * P, :].rearrange("(kb p) c -> p kb c", p=P),
            )
            dma_x_batch("xv", xv, 0, 0)
            dma_x_batch("xv", xv, 0, 1)
            nc.sync.dma_start(tri_sb[:], tri[:])
            # preload the Exp table on ScalarE before attention needs it
            nc.any.memset(warm_sb[:], 1.0)
            nc.scalar.activation(
                warm_sb[:], warm_sb[:], mybir.ActivationFunctionType.Exp, scale=SCALE
            )

            # ---- per-chunk emission bodies ----
            def proj_step(j):
                jsl = slice(j * TJ, (j + 1) * TJ)
                # K then Q, channel-major [256ch, t]
                for nm, w_sb, dst, bi in (
                    ("xk", wk_sb, kt_sb, 1),
                    ("xq", wq_sb, qt_sb, 0),
                ):
                    for mc in range(2):
                        ps = pspool.tile([P, TJ], F32, tag="ps_pr", name="ps_pr", bufs=2)
                        for kb in range(KB):
                            nc.tensor.matmul(
                                ps[:],
                                lhsT=w_sb[:, kb, mc * P : (mc + 1) * P],
                                rhs=xt[(nm, kb, j)][:],
                                start=(kb == 0),
                                stop=(kb == KB - 1),
                            )
                        nc.vector.tensor_scalar_add(
                            dst[:, mc, jsl],
                            ps[:],
                            padb_sb[:, NSB + 2 * bi + mc : NSB + 2 * bi + mc + 1],
                        )
                # V, sequence-major [s, 260] with ones column per head
                for ii in range(TJ // P):
                    i = j * (TJ // P) + ii
                    ps = pspool.tile([P, TJ], F32, tag="ps_pr", name="ps_v", bufs=2)
                    for kb in range(KB):
                        nc.tensor.matmul(
                            ps[:, :VC],
                            lhsT=xt[("xv", kb, j)][:, ii * P : (ii + 1) * P],
                            rhs=wv_sb[:, kb, :],
                            start=(kb == 0),
                            stop=(kb == KB - 1),
                        )
                    vb = npool.tile([P, VC], F32, tag="vb", name="vb", bufs=3)
                    nc.vector.tensor_tensor(
                        out=vb[:],
                        in0=ps[:, :VC],
                        in1=padb_sb[:, NSB + 4 :],
                        op=mybir.AluOpType.add,
                    )
                    nc.vector.tensor_scalar_mul(
                        v_sb[:, i, :], vb[:], padb_sb[:, i : i + 1]
                    )

            def attn_chunk(hp, j, aoTn):
                """Causal attention for head pair hp over t-chunk j.

                Boundary s-blocks (i = 4j..4j+3, r = i-4j) are narrowed to
                cols >= 128*r of the chunk on scores/exp/attnV; the diagonal
                128x128 triangle is masked with a fp16 multiply on DVE.
                """
                nsb_j = 4 * j + 4
                jsl = slice(j * TJ, (j + 1) * TJ)
                av_ps = [
                    pspool.tile([P, TJ], F32, tag=f"ps_av{lh}", name="ps_av", bufs=1)
                    for lh in range(2)
                ]
                ets = {}

                def emit_scores_pair(m):
                    ps2s = [
                        pspool.tile([P, 2, TJ], F32, tag="ps_sc", name="ps_sc", bufs=2)
                        for _ in range(2)
                    ]
                    for u in range(2):
                        i = 2 * m + u
                        r = i - 4 * j
                        off = 128 * r if r > 0 else 0
                        for lh in range(2):
                            base = D * lh
                            nc.tensor.matmul(
                                ps2s[lh][:, u, off:],
                                lhsT=kt_sb[base : base + D, hp, i * P : (i + 1) * P],
                                rhs=qt_sb[base : base + D, hp, j * TJ + off : (j + 1) * TJ],
                                start=True,
                                stop=True,
                            )
                    r1 = 2 * m + 1 - 4 * j  # r of u=1; >0 means narrowed pair
                    for lh in range(2):
                        et2 = epool.tile([P, 2, TJ], DT, tag="exp", name="et2")
                        if r1 <= 0:
                            nc.scalar.activation(
                                et2[:],
                                ps2s[lh][:],
                                mybir.ActivationFunctionType.Exp,
                                scale=SCALE,
                            )
                        else:
                            for u in range(2):
                                off = 128 * (r1 - 1 + u)
                                nc.scalar.activation(
                                    et2[:, u, off:],
                                    ps2s[lh][:, u, off:],
                                    mybir.ActivationFunctionType.Exp,
                                    scale=SCALE,
                                )
                        for u in range(2):
                            r = 2 * m + u - 4 * j
                            if r >= 0:
                                # diagonal block: mask the 128x128 triangle
                                nc.vector.tensor_mul(
                                    out=et2[:, u, 128 * r : 128 * (r + 1)],
                                    in0=et2[:, u, 128 * r : 128 * (r + 1)],
                                    in1=tri_sb[:],
                                )
                        ets[(m, lh)] = et2

                def emit_av_pair(m, lh):
                    h65 = (hp * 2 + lh) * (D + 1)
                    et2 = ets.pop((m, lh))
                    for u in range(2):
                        i = 2 * m + u
                        r = i - 4 * j
                        off = 128 * r if r > 0 else 0
                        nc.tensor.matmul(
                            av_ps[lh][: D + 1, off:],
                            lhsT=v_sb[:, i, h65 : h65 + D + 1],
                            rhs=et2[:, u, off:],
                            start=(i == 0),
                            stop=(i == nsb_j - 1),
                            skip_group_check=True,
                        )

                # software pipeline: attn@V trails scores/exp by one pair
                npairs = nsb_j // 2
                for m in range(npairs):
                    emit_scores_pair(m)
                    if m >= 1:
                        for lh in range(2):
                            emit_av_pair(m - 1, lh)
                for lh in range(2):
                    emit_av_pair(npairs - 1, lh)
                # normalize: den sits in row D of av_ps; reciprocal in place,
                # broadcast to 64 partitions, multiply.  lh=1 lands on
                # partitions 64-127 of aoTn via an SBUF->SBUF shift DMA.
                for lh in range(2):
                    aoTS = apool.tile(
                        [D + 1, TJ], F32, tag=f"aoTS{lh}", name="aoTS", bufs=2
                    )
                    nc.any.tensor_copy(out=aoTS[:], in_=av_ps[lh][: D + 1, :])
                    rcp = npool.tile([1, TJ], F32, tag="rcp", name="rcp", bufs=3)
                    nc.gpsimd.dma_start(rcp[:], aoTS[D : D + 1, :])
                    nc.vector.reciprocal_approx_fast(rcp[:], rcp[:])
                    rb64 = npool.tile([D, TJ], F32, tag="rb64", name="rb64", bufs=3)
                    nc.gpsimd.partition_broadcast(rb64[:], rcp[0:1, :])
                    if lh == 0:
                        nc.vector.tensor_mul(
                            out=aoTn[0:D, :], in0=aoTS[0:D, :], in1=rb64[:]
                        )
                    else:
                        tmp = npool.tile([D, TJ], DT, tag="aon", name="aon", bufs=3)
                        nc.vector.tensor_mul(
                            out=tmp[:], in0=aoTS[0:D, :], in1=rb64[:]
                        )
                        # partition shift 0-63 -> 64-127 via SBUF DMA
                        nc.gpsimd.dma_start(aoTn[D : 2 * D, :], tmp[:])

            def oproj_step(j, aoTn0, aoTn1):
                jsl = slice(j * TJ, (j + 1) * TJ)
                for mc in range(KB):
                    ps = pspool.tile([P, TJ], F32, tag="ps_pr", name="ps_o", bufs=2)
                    for cc, src in ((0, aoTn0), (1, aoTn1)):
                        nc.tensor.matmul(
                            ps[:],
                            lhsT=wo_sb[:, cc, mc * P : (mc + 1) * P],
                            rhs=src[:],
                            start=(cc == 0),
                            stop=(cc == 1),
                        )
                    oc = fpool.tile([P, TJ], F16, tag="oc", name="oc", bufs=4)
                    nc.any.tensor_copy(out=oc[:], in_=ps[:])
                    eng = nc.gpsimd if mc % 2 == 0 else nc.sync
                    eng.dma_start(out_t[mc * P : (mc + 1) * P, jsl], oc[:])

            # ---- main pipeline ----
            proj_step(0)
            for j in range(NTJ):
                if j + 1 < NTJ:
                    dma_x_step(j + 1)
                if j == 0:
                    nc.sync.dma_start(wo_sb[:], wo_r)
                aoTns = []
                for hp in range(2):
                    aoTn = npool.tile([P, TJ], DT, tag=f"aoTn{hp}", name="aoTn", bufs=2)
                    attn_chunk(hp, j, aoTn)
                    aoTns.append(aoTn)
                if j + 1 < NTJ:
                    proj_step(j + 1)
                oproj_step(j, aoTns[0], aoTns[1])

    nc.compile()
    return nc


_NC_CACHE = None


def _get_nc():
    global _NC_CACHE
    if _NC_CACHE is None:
        _NC_CACHE = _build_program()
    return _NC_CACHE


def _make_in_maps(query, key, value, key_padding_mask, Wq, bq, Wk, bk, Wv, bv, Wo, bo):
    f32 = np.float32
    query = np.asarray(query, f32)
    key = np.asarray(key, f32)
    value = np.asarray(value, f32)
    kpm = np.asarray(key_padding_mask, bool)
    Wq, bq = np.asarray(Wq, f32), np.asarray(bq, f32)
    Wk, bk = np.asarray(Wk, f32), np.asarray(bk, f32)
    Wv, bv = np.asarray(Wv, f32), np.asarray(bv, f32)
    Wo = np.asarray(Wo, f32)

    # constants shared by all cores
    f16 = np.float16
    tri_np = (np.arange(P)[None, :] >= np.arange(P)[:, None]).astype(f16)

    in_maps = []
    for c in range(8):
        b, g = divmod(c, 4)
        cols = slice(g * GC, (g + 1) * GC)

        wq_t = Wq[cols, :].T.astype(f16)
        wk_t = Wk[cols, :].T.astype(f16)

        wv_t = np.zeros((E + 1, VC), f16)
        for h in range(NHL):
            ch = slice(g * GC + h * D, g * GC + (h + 1) * D)
            wv_t[:E, h * (D + 1) : h * (D + 1) + D] = Wv[ch, :].T
            wv_t[E, h * (D + 1) : h * (D + 1) + D] = bv[ch]
            wv_t[E, h * (D + 1) + D] = 1.0  # ones column -> softmax denominator

        wo_t = np.ascontiguousarray(Wo[:, cols].T.astype(f16))

        padb_np = np.where(kpm[b], 0.0, 1.0).astype(f32).reshape(NSB, P).T
        biases = np.stack(
            [bq[cols][:P], bq[cols][P:], bk[cols][:P], bk[cols][P:]], axis=1
        ).astype(f32)
        vbias = np.tile(wv_t[E].astype(f32)[None, :], (P, 1))
        padb_np = np.ascontiguousarray(
            np.concatenate([padb_np, biases, vbias], axis=1)
        )

        in_maps.append(
            {
                "xq_t": np.ascontiguousarray(query[b].T.astype(f16)),
                "xk_t": np.ascontiguousarray(key[b].T.astype(f16)),
                "xv_t": np.ascontiguousarray(value[b].T.astype(f16)),
                "wq_t": np.ascontiguousarray(wq_t),
                "wk_t": np.ascontiguousarray(wk_t),
                "wv_t": wv_t,
                "wo_t": wo_t,
                "tri": tri_np,
                "padb": padb_np,
            }
        )
    return in_maps


def kernel(**inputs) -> np.ndarray:
    nc = _get_nc()
    in_maps = _make_in_maps(**inputs)
    res = run_bass_kernel_spmd(nc, in_maps, core_ids=list(range(8)))
    bo = np.asarray(inputs["bo"], np.float32)
    B = inputs["query"].shape[0]
    out = np.zeros((B, T, E), np.float32)
    for c in range(8):
        b = c // 4
        out[b] += res.results[c]["out_t"].T.astype(np.float32)
    out += bo[None, None, :]
    return out
